# revision 22
# baseline (speedup 1.0000x reference)
"""DOS loss kernel for Trainium2, 8 NeuronCores, SPMD, collective-free.

loss = sum(w * d) + sum(softmax(-w * d, axis=-1) @ ce)
  d[k]  = ||deep_feats - n[k]||_2                      (K)
  ce[k] = logsumexp(cls_score[k]) - cls_score[k, tgt]  (K)

Sharding: the K (contraction) dimension is split 512/core everywhere —
n rows, cls rows, and a [512, W] slice of (-w)^T (host-transposed and
HOST-NEGATED so the device only ever needs +d). Each core computes its
local d/ce shard plus partial stats over the full W:
  s_row[r]   += sum_{k in shard} exp(-d_k w[r,k])
  num_row[r] += sum_{k in shard} ce_k exp(-d_k w[r,k])
  f_row[r]   += sum_{k in shard} -d_k w[r,k]
Each core DMAs its [4, W] partial out; the host completes the
reduction (loss = sum_r Num/S - sum F). No on-device collective.

Numerics: w, cls, exp tiles, ce and the split d ride fp8e4; n and deep
are bf16. d rides the f matmul as a split fp8 pair d = d_hi + d_lo.

Changes vs the 54.8us baseline (each validated on a perfetto trace):
 - d^2 per chunk = DVE sub + 4x bn_stats[128,512] + bn_aggr
   (sum x^2 = 2048*(var+mean^2)) instead of sub+mul+reduce: ~1us less
   DVE serial time per chunk, so every d_t lands just before the ACT
   exp stream needs it (ACT busy time is the kernel's floor).
 - d = exp(0.5*ln(2048*(var+mean^2))) keeps the single Exp/Ln/Copy ACT
   table set; w is pre-negated on the host so d stays positive and no
   negate op exists anywhere.
 - DMA plan (all rows 4KB; half-width rows halve a queue's rate):
   n0 leads sync, deep leads scalar, cls leads gpsimd, so sub0 starts
   ~12.5us and the cls block starts when cls lands (~15.5); the two
   first-wave chains (d-chain and cls block) converge on the first
   wexp at ~21 with no ACT bubble. Tiny DMAs never sit in front of a
   critical one: each dma_start costs ~1.3-2.4us of queue dead time.
 - f matmuls all accumulate into ONE [2, 512] PSUM region (the host
   sums those 512 cols), shrinking the f tail copy from 4096 to 512
   columns.
 - wexp chunks 2/3 are split into 2/3 pieces so the tail DR matmuls
   and the PSUM->SBUF quarter-copies chase individual banks.
 - All tiny fixups (ce add, sn2 build, fp8 d split, d^2 assembly) run
   on the otherwise-idle GpSimd.
 - tc.tile_wait_until stamps keep the static scheduler from parking
   DMA-gated ops (sub_t, stage-B gpsimd ops) ahead of ready d-chain
   tail ops on the in-order engines -- the v2-v4 traces all showed
   multi-us head-of-line stalls from exactly that.
"""

import sys

import numpy as np

for _p in ("/opt/trn_rl_repo",):
    if _p not in sys.path:
        sys.path.insert(0, _p)

D, K, W, C = 2048, 4096, 4096, 1000
NCORES = 8
KS = K // NCORES  # 512 k rows per core
KT = KS // 128  # 4 k chunks per core
NP = KT // 2  # chunk pairs (DoubleRow contracts 2 chunks per pass)
NB = W // 512  # 8 psum bank slices

_STATE = None


def _build():
    import math
    import types

    import concourse.bass as bass
    from concourse import bacc, mybir, tile
    from concourse.hw_specs import get_activation_tables

    F32 = mybir.dt.float32
    BF16 = mybir.dt.bfloat16
    FP8 = mybir.dt.float8e4
    AF = mybir.ActivationFunctionType
    OP = mybir.AluOpType
    DR = mybir.MatmulPerfMode.DoubleRow

    nc = bacc.Bacc("TRN2", target_bir_lowering=False, debug=False, num_devices=NCORES)

    # Route every Exp/Ln/Copy activation to the one table set that has
    # all three, so only a single ACT_TABLE_LOAD is ever emitted.
    _KEEP = {AF.Exp, AF.Ln, AF.Copy}
    _HOME = "natural_log_exp_and_others"

    def _one_table_set(self):
        has_activation = any(
            isinstance(i, mybir.InstActivation)
            for b in self.main_func.blocks
            for i in b.instructions
        )
        if not has_activation:
            return
        tables = [
            (name, fns if name == _HOME else (fns - _KEEP))
            for name, fns in get_activation_tables(self.m.arch).items()
        ]
        mybir._bass_rust.insert_act_table_loads(self, tables)

    nc.insert_act_table_loads = types.MethodType(_one_table_set, nc)

    deep_d = nc.dram_tensor("deep", [128, D], BF16, kind="ExternalInput")
    n_d = nc.dram_tensor("n_s", [KS, D], BF16, kind="ExternalInput")
    cls_d = nc.dram_tensor("cls_s", [128, KT * C], FP8, kind="ExternalInput")
    ncol_d = nc.dram_tensor("ncol_s", [KS], F32, kind="ExternalInput")
    wt_d = nc.dram_tensor("wt_s", [KS, W], FP8, kind="ExternalInput")  # = -w^T
    out_d = nc.dram_tensor("out", [4, W], F32, kind="ExternalOutput")



    with tile.TileContext(nc) as tc:
        with (
            tc.tile_pool(name="small", bufs=1) as sm,
            tc.tile_pool(name="npool", bufs=4) as npool,
            tc.tile_pool(name="nscr", bufs=2) as nscr,
            tc.tile_pool(name="clsscr", bufs=2) as clsscr,
            tc.tile_pool(name="psum", bufs=1, space="PSUM") as pp,
        ):
            # Warm the exp/ln table set immediately, from a const input
            # so no memset/DMA gates the ACT_TABLE_LOAD.
            warm = sm.tile([1, 1], F32)
            nc.scalar.activation(
                warm[:], nc.const_aps.scalar_like(1.0, warm[:])[0:1, :], AF.Exp
            )

            # ---------------- input loads ----------------------------
            # Three DMA queues (sync/scalar = HWDGE, gpsimd = SWDGE).
            # DMA rows must stay 4KB: narrower rows halve a queue's
            # rate. Queue spin-ups are ~8.5/9.5/11us, so the three
            # first-wave tensors ride one front each.
            n_ts = [npool.tile([128, D], BF16, name=f"n_{t}") for t in range(KT)]
            clsb = sm.tile([128, KT, C], FP8)
            ncol_sb = sm.tile([128, KT], F32)
            w2s = [
                sm.tile([128, 2, W], FP8, tag=f"w2_{p}", name=f"w2_{p}")
                for p in range(NP)
            ]
            deep_b = sm.tile([128, D], BF16, name="deep_b")
            # First-wave fronts: n0 on sync (fastest spin-up), deep
            # (host-broadcast [128, D]) on scalar, cls on gpsimd. d0 is
            # then ready ~17-19 while the cls block runs 13.6-19.5 --
            # jointly near the ACT-work conservation floor.
            # sync: n0, n1 (sync drains by ~13.5, freeing fabric for cls)
            nc.sync.dma_start(n_ts[0][:], n_d[0:128, :])
            nc.sync.dma_start(n_ts[1][:], n_d[128:256, :])
            # scalar: deep, n2, n3, ncol
            nc.scalar.dma_start(deep_b[:], deep_d[:])
            nc.scalar.dma_start(n_ts[2][:], n_d[256:384, :])
            nc.scalar.dma_start(n_ts[3][:], n_d[384:512, :])
            nc.scalar.dma_start(
                ncol_sb[:], ncol_d[:].rearrange("(t p) -> p t", p=128)
            )
            # gpsimd: cls, then the four w chunks
            nc.gpsimd.dma_start(clsb[:], cls_d[:])
            for t in range(KT):
                nc.gpsimd.dma_start(
                    w2s[t // 2][:, t % 2, :], wt_d[t * 128 : (t + 1) * 128, :]
                )

            sn_psum = pp.tile([128, W], F32, tag="ps")

            # ------------- stage A: local d, per chunk ----------------
            # diff = n - deep (DVE TT bf16 2x), then 4x bn_stats over
            # 512-groups + bn_aggr: mean/var of the 2048 dims. GpSimd
            # assembles s2 = var + mean^2; ACT does d = exp(.5 ln s2 +
            # .5 ln 2048). The fp8 hi/lo split for the f matmul runs on
            # GpSimd into fd2[p][:, c, 0:2] (0=hi, 1=lo).
            stats = sm.tile([128, KT, 4, 6], F32)
            mv = sm.tile([128, KT, 2], F32)
            msq = sm.tile([128, KT], F32)
            s2col = sm.tile([128, KT], F32)
            lnd2 = sm.tile([128, KT], F32)
            dcol = sm.tile([128, KT], F32)
            dh32 = sm.tile([128, KT], F32)
            dlo = sm.tile([128, KT], F32)
            fd2s = [
                sm.tile([128, 2, 16], FP8, tag=f"fd2_{p}", name=f"fd2_{p}")
                for p in range(NP)
            ]
            for t in range(KT):
                diff = nscr.tile([128, D], BF16, tag="ascr")
                # Manual wait stamp: keep sub_t from being scheduled
                # ahead of chunk t-1's bn/aggr tail when the scheduler
                # mispredicts the n_t DMA (v2/v3 traces both showed the
                # DVE head-of-line blocked on a not-yet-landed n chunk).
                with tc.tile_wait_until(0.012 + 0.0042 * t):
                    nc.vector.tensor_sub(diff[:], n_ts[t][:], deep_b[:])
                for g in range(4):
                    nc.vector.bn_stats(
                        stats[:, t, g, :], diff[:, g * 512 : (g + 1) * 512]
                    )
                nc.vector.bn_aggr(mv[:, t, :], stats[:, t, :, :])
                # GpSimd: s2 = var + mean^2 (stamped: the v4 trace had
                # the scheduler park stage-B gpsimd ops ahead of these,
                # delaying d0 by ~7us)
                with tc.tile_wait_until(0.013 + 0.0042 * t):
                    nc.gpsimd.tensor_tensor(
                        msq[:, t : t + 1], mv[:, t, 0:1], mv[:, t, 0:1], OP.mult
                    )
                    nc.gpsimd.tensor_tensor(
                        s2col[:, t : t + 1], msq[:, t : t + 1], mv[:, t, 1:2], OP.add
                    )
                # ACT: d = sqrt(2048 * s2) = exp(0.5 ln(2048 * s2))
                nc.scalar.activation(
                    lnd2[:, t : t + 1], s2col[:, t : t + 1], AF.Ln, scale=float(D)
                )
                nc.scalar.activation(
                    dcol[:, t : t + 1], lnd2[:, t : t + 1], AF.Exp, scale=0.5
                )
                # GpSimd: fp8 hi/lo split (d is positive; w is -w)
                fd2 = fd2s[t // 2]
                c = t % 2
                with tc.tile_wait_until(0.0135 + 0.0042 * t):
                    nc.gpsimd.tensor_copy(fd2[:, c, 0:1], dcol[:, t : t + 1])
                    nc.gpsimd.tensor_copy(dh32[:, t : t + 1], fd2[:, c, 0:1])
                    nc.gpsimd.tensor_tensor(
                        dlo[:, t : t + 1],
                        dcol[:, t : t + 1],
                        dh32[:, t : t + 1],
                        OP.subtract,
                    )
                    nc.gpsimd.tensor_copy(fd2[:, c, 1:2], dlo[:, t : t + 1])

            # ---------------- stage B: local ce -----------------------
            ssum = sm.tile([128, KT], F32)
            for t in range(KT):
                escr = clsscr.tile([128, C], BF16, tag="bscr")
                nc.scalar.activation(
                    escr[:], clsb[:, t, :], AF.Exp, accum_out=ssum[:, t : t + 1]
                )
            lse = sm.tile([128, KT], F32)
            nc.scalar.activation(lse[:], ssum[:], AF.Ln)
            cecol = sm.tile([128, KT], F32)
            with tc.tile_wait_until(0.026):
                nc.gpsimd.tensor_tensor(cecol[:], lse[:], ncol_sb[:], OP.add)
            # DoubleRow lhsT pairs [ones | ce] per chunk pair, fp8
            sn2s = []
            for p in range(NP):
                sn2 = sm.tile([128, 2, 16], FP8, tag=f"sn2_{p}")
                nc.gpsimd.memset(sn2[:, :, 0:1], 1.0)
                with tc.tile_wait_until(0.0265):
                    nc.gpsimd.tensor_copy(
                        sn2[:, :, 1:2], cecol[:, 2 * p : 2 * p + 2]
                    )
                sn2s.append(sn2)

            # ------- stage C: sweep local wT over all W ---------------
            # One [34, W] f32 PSUM tile: rows 0-1 = [s, num] (DoubleRow,
            # must land at partition 0), rows 32-33 = [-f_hi, -f_lo]
            # (regular fp8, FD=1024 passes). exp pieces for chunks 2/3
            # are split so tail DR matmuls chase individual banks.
            e2s = [sm.tile([128, 2, W], FP8, tag=f"e2_{p}", name=f"e2_{p}") for p in range(NP)]

            # exp piece lists per chunk: (pair, c, lo, hi)
            # chunk 3 is split so the tail DR matmuls chase its pieces;
            # chunk 2 is NOT split: DR pair-B banks are gated by chunk
            # 3's pieces, which the in-order ACT engine runs after all
            # of chunk 2 anyway, so a c2 split is pure overhead.
            pieces = {
                0: [(0, 0, 0, W)],
                1: [(0, 1, 0, W)],
                2: [(1, 0, 0, W)],
                3: [(1, 1, 0, 2048), (1, 1, 2048, 3072), (1, 1, 3072, W)],
            }
            for t in range(KT):
                for p, c, lo, hi in pieces[t]:
                    nc.scalar.activation(
                        e2s[p][:, c, lo:hi],
                        w2s[p][:, c, lo:hi],
                        AF.Exp,
                        scale=dcol[:, t : t + 1],
                    )

            # f matmuls: regular fp8, 512-wide passes, ALL accumulating
            # into the SAME [2, 512] psum region (host sums the 512
            # cols) so the tail copy is 512 cols instead of 4096.
            def f_mms(p, c):
                t = p * 2 + c
                for b in range(NB):
                    sl = slice(b * 512, (b + 1) * 512)
                    nc.tensor.matmul(
                        sn_psum[32:34, 0:512],
                        fd2s[p][:, c, 0:2],
                        w2s[p][:, c, sl],
                        start=(t == 0 and b == 0),
                        stop=(t == KT - 1 and b == NB - 1),
                        skip_group_check=True,
                    )

            def dr_mms(p, blo, bhi):
                for b in range(blo, bhi):
                    sl = slice(b * 512, (b + 1) * 512)
                    nc.tensor.matmul(
                        sn_psum[0:2, sl],
                        sn2s[p][:, :, 0:2],
                        e2s[p][:, :, sl],
                        start=(p == 0),
                        stop=(p == NP - 1),
                        perf_mode=DR,
                        skip_group_check=True,
                    )

            # PE program order: f chunks 0-2 early, DR pair A after
            # wexp1, f chunk 3 (gated on d3) overlapping wexp3, then DR
            # pair B banks chasing the wexp3 pieces.
            f_mms(0, 0)
            f_mms(0, 1)
            f_mms(1, 0)
            dr_mms(0, 0, NB)
            f_mms(1, 1)
            dr_mms(1, 0, 4)
            dr_mms(1, 4, 6)
            dr_mms(1, 6, 8)

            # PSUM -> SBUF (DMA cannot read PSUM). f finishes first ->
            # DVE takes its low half early, ACT the high half right
            # after the exp stream ends. sn copies chase DR banks in
            # quarters, split DVE/ACT.
            f_sb = sm.tile([2, 512], F32)
            nc.vector.tensor_copy(f_sb[:], sn_psum[32:34, 0:512])
            nc.sync.dma_start(out_d[2:4, 0:512], f_sb[:])
            sn_sb = sm.tile([2, W], F32)
            Q = W // 4
            nc.vector.tensor_copy(sn_sb[:, 0:Q], sn_psum[0:2, 0:Q])
            nc.vector.tensor_copy(sn_sb[:, Q : 2 * Q], sn_psum[0:2, Q : 2 * Q])
            nc.scalar.copy(sn_sb[:, 2 * Q : 3 * Q], sn_psum[0:2, 2 * Q : 3 * Q])
            nc.scalar.copy(sn_sb[:, 3 * Q : W], sn_psum[0:2, 3 * Q : W])
            nc.sync.dma_start(out_d[0:2, :], sn_sb[:])

    nc.compile()
    return nc


def _get_state():
    global _STATE
    if _STATE is None:
        _STATE = _build()
    return _STATE


def _shard_inputs(deep_feats, cls_score, target, n, w):
    import ml_dtypes

    bf16 = ml_dtypes.bfloat16
    fp8 = ml_dtypes.float8_e4m3
    deep_feats = np.ascontiguousarray(deep_feats, dtype=np.float32).reshape(1, D)
    cls_score = np.ascontiguousarray(cls_score, dtype=np.float32)
    n = np.ascontiguousarray(n, dtype=np.float32)
    w = np.ascontiguousarray(w, dtype=np.float32)
    tgt = int(np.asarray(target).reshape(-1)[0])
    ncol = -cls_score[:, tgt].astype(np.float32)

    deep_b = np.ascontiguousarray(np.broadcast_to(deep_feats.astype(bf16), (128, D)))
    n_bf = n.astype(bf16)
    cls_8 = cls_score.astype(fp8)
    wt_8 = np.ascontiguousarray((-w.T).astype(fp8))  # [K, W], negated

    in_maps = []
    for i in range(NCORES):
        ks = slice(i * KS, (i + 1) * KS)
        # cls reshaped so SBUF partition rows are KT*C bytes (4KB DMA
        # rows): cls_r[p, t*C + c] = cls[ks][t*128+p, c]
        cls_r = np.ascontiguousarray(
            cls_8[ks].reshape(KT, 128, C).transpose(1, 0, 2).reshape(128, KT * C)
        )
        in_maps.append(
            {
                "deep": deep_b,
                "n_s": n_bf[ks],
                "cls_s": cls_r,
                "ncol_s": ncol[ks],
                "wt_s": wt_8[ks],
            }
        )
    return in_maps


def _combine(outs):
    """Host-side unshard: sum the 8 [4, W] partials and finish the loss."""
    acc = np.zeros((4, W), dtype=np.float64)
    for o in outs:
        acc += np.asarray(o, dtype=np.float64)
    s_row, num_row = acc[0], acc[1]
    g = float(np.sum(num_row / s_row))
    f = -float(np.sum(acc[2, :512] + acc[3, :512]))  # rows hold -d*w partials
    return np.float32(g + f).reshape(())


def kernel(deep_feats, cls_score, target, n, w):
    nc = _get_state()
    from concourse.bass_utils import run_bass_kernel_spmd

    in_maps = _shard_inputs(deep_feats, cls_score, target, n, w)
    res = run_bass_kernel_spmd(nc, in_maps, list(range(NCORES)))
    return _combine([res.results[i]["out"] for i in range(NCORES)])


# revision 23
# speedup vs baseline: 1.0388x; 1.0388x over previous
"""DOS loss kernel for Trainium2, 8 NeuronCores, SPMD, collective-free.

loss = sum(w * d) + sum(softmax(-w * d, axis=-1) @ ce)
  d[k]  = ||deep_feats - n[k]||_2                      (K)
  ce[k] = logsumexp(cls_score[k]) - cls_score[k, tgt]  (K)

Sharding: the K (contraction) dimension is split 512/core everywhere —
n rows, cls rows, and a [512, W] slice of (-w)^T (host-transposed and
HOST-NEGATED so the device only ever needs +d). Each core computes its
local d/ce shard plus partial stats over the full W:
  s_row[r]   += sum_{k in shard} exp(-d_k w[r,k])
  num_row[r] += sum_{k in shard} ce_k exp(-d_k w[r,k])
  f_row[r]   += sum_{k in shard} -d_k w[r,k]
Each core DMAs its [4, W] partial out; the host completes the
reduction (loss = sum_r Num/S - sum F). No on-device collective.

Numerics: w, cls, exp tiles, ce and the split d ride fp8e4; n and deep
are bf16. d rides the f matmul as a split fp8 pair d = d_hi + d_lo.

Changes vs the 54.8us baseline (each validated on a perfetto trace):
 - d^2 per chunk = DVE sub + 4x bn_stats[128,512] + bn_aggr
   (sum x^2 = 2048*(var+mean^2)) instead of sub+mul+reduce: ~1us less
   DVE serial time per chunk, so every d_t lands just before the ACT
   exp stream needs it (ACT busy time is the kernel's floor).
 - d = exp(0.5*ln(2048*(var+mean^2))) keeps the single Exp/Ln/Copy ACT
   table set; w is pre-negated on the host so d stays positive and no
   negate op exists anywhere.
 - DMA plan (all rows 4KB; half-width rows halve a queue's rate):
   n0 leads sync, deep leads scalar, cls leads gpsimd, so sub0 starts
   ~12.5us and the cls block starts when cls lands (~15.5); the two
   first-wave chains (d-chain and cls block) converge on the first
   wexp at ~21 with no ACT bubble. Tiny DMAs never sit in front of a
   critical one: each dma_start costs ~1.3-2.4us of queue dead time.
 - f matmuls all accumulate into ONE [2, 512] PSUM region (the host
   sums those 512 cols), shrinking the f tail copy from 4096 to 512
   columns.
 - wexp chunks 2/3 are split into 2/3 pieces so the tail DR matmuls
   and the PSUM->SBUF quarter-copies chase individual banks.
 - All tiny fixups (ce add, sn2 build, fp8 d split, d^2 assembly) run
   on the otherwise-idle GpSimd.
 - tc.tile_wait_until stamps keep the static scheduler from parking
   DMA-gated ops (sub_t, stage-B gpsimd ops) ahead of ready d-chain
   tail ops on the in-order engines -- the v2-v4 traces all showed
   multi-us head-of-line stalls from exactly that.
"""

import sys

import numpy as np

for _p in ("/opt/trn_rl_repo",):
    if _p not in sys.path:
        sys.path.insert(0, _p)

D, K, W, C = 2048, 4096, 4096, 1000
NCORES = 8
KS = K // NCORES  # 512 k rows per core
KT = KS // 128  # 4 k chunks per core
NP = KT // 2  # chunk pairs (DoubleRow contracts 2 chunks per pass)
NB = W // 512  # 8 psum bank slices

_STATE = None


def _build():
    import math
    import types

    import concourse.bass as bass
    from concourse import bacc, mybir, tile
    from concourse.hw_specs import get_activation_tables

    F32 = mybir.dt.float32
    BF16 = mybir.dt.bfloat16
    FP8 = mybir.dt.float8e4
    AF = mybir.ActivationFunctionType
    OP = mybir.AluOpType
    DR = mybir.MatmulPerfMode.DoubleRow

    nc = bacc.Bacc("TRN2", target_bir_lowering=False, debug=False, num_devices=NCORES)

    # Route every Exp/Ln/Copy activation to the one table set that has
    # all three, so only a single ACT_TABLE_LOAD is ever emitted.
    _KEEP = {AF.Exp, AF.Ln, AF.Copy}
    _HOME = "natural_log_exp_and_others"

    def _one_table_set(self):
        has_activation = any(
            isinstance(i, mybir.InstActivation)
            for b in self.main_func.blocks
            for i in b.instructions
        )
        if not has_activation:
            return
        tables = [
            (name, fns if name == _HOME else (fns - _KEEP))
            for name, fns in get_activation_tables(self.m.arch).items()
        ]
        mybir._bass_rust.insert_act_table_loads(self, tables)

    nc.insert_act_table_loads = types.MethodType(_one_table_set, nc)

    deep_d = nc.dram_tensor("deep", [128, D], BF16, kind="ExternalInput")
    n_d = nc.dram_tensor("n_s", [KS, D], BF16, kind="ExternalInput")
    cls_d = nc.dram_tensor("cls_s", [128, KT * C], FP8, kind="ExternalInput")
    ncol_d = nc.dram_tensor("ncol_s", [KS], F32, kind="ExternalInput")
    wt_d = nc.dram_tensor("wt_s", [KS, W], FP8, kind="ExternalInput")  # = -w^T
    out_d = nc.dram_tensor("out", [4, W], F32, kind="ExternalOutput")



    with tile.TileContext(nc) as tc:
        with (
            tc.tile_pool(name="small", bufs=1) as sm,
            tc.tile_pool(name="npool", bufs=4) as npool,
            tc.tile_pool(name="nscr", bufs=2) as nscr,
            tc.tile_pool(name="clsscr", bufs=2) as clsscr,
            tc.tile_pool(name="psum", bufs=1, space="PSUM") as pp,
        ):
            # Warm the exp/ln table set immediately, from a const input
            # so no memset/DMA gates the ACT_TABLE_LOAD.
            warm = sm.tile([1, 1], F32)
            nc.scalar.activation(
                warm[:], nc.const_aps.scalar_like(1.0, warm[:])[0:1, :], AF.Exp
            )

            # ---------------- input loads ----------------------------
            # Three DMA queues (sync/scalar = HWDGE, gpsimd = SWDGE).
            # DMA rows must stay 4KB: narrower rows halve a queue's
            # rate. Queue spin-ups are ~8.5/9.5/11us, so the three
            # first-wave tensors ride one front each.
            n_ts = [npool.tile([128, D], BF16, name=f"n_{t}") for t in range(KT)]
            clsb = sm.tile([128, KT, C], FP8)
            ncol_sb = sm.tile([128, KT], F32)
            w2s = [
                sm.tile([128, 2, W], FP8, tag=f"w2_{p}", name=f"w2_{p}")
                for p in range(NP)
            ]
            deep_b = sm.tile([128, D], BF16, name="deep_b")
            # First-wave fronts: n0 on sync (fastest spin-up), deep
            # (host-broadcast [128, D]) on scalar, cls on gpsimd. d0 is
            # then ready ~17-19 while the cls block runs 13.6-19.5 --
            # jointly near the ACT-work conservation floor.
            # sync: n0, n1 (sync drains by ~13.5, freeing fabric for cls)
            nc.sync.dma_start(n_ts[0][:], n_d[0:128, :])
            nc.sync.dma_start(n_ts[1][:], n_d[128:256, :])
            # scalar: deep, n2, n3, ncol
            nc.scalar.dma_start(deep_b[:], deep_d[:])
            nc.scalar.dma_start(n_ts[2][:], n_d[256:384, :])
            nc.scalar.dma_start(n_ts[3][:], n_d[384:512, :])
            nc.scalar.dma_start(
                ncol_sb[:], ncol_d[:].rearrange("(t p) -> p t", p=128)
            )
            # gpsimd: cls, then the four w chunks
            nc.gpsimd.dma_start(clsb[:], cls_d[:])
            for t in range(KT):
                nc.gpsimd.dma_start(
                    w2s[t // 2][:, t % 2, :], wt_d[t * 128 : (t + 1) * 128, :]
                )

            sn_psum = pp.tile([128, W], F32, tag="ps")

            # ------------- stage A: local d, per chunk ----------------
            # diff = n - deep (DVE TT bf16 2x), then 4x bn_stats over
            # 512-groups + bn_aggr: mean/var of the 2048 dims. GpSimd
            # assembles s2 = var + mean^2; ACT does d = exp(.5 ln s2 +
            # .5 ln 2048). The fp8 hi/lo split for the f matmul runs on
            # GpSimd into fd2[p][:, c, 0:2] (0=hi, 1=lo).
            stats = sm.tile([128, KT, 4, 6], F32)
            mv = sm.tile([128, KT, 2], F32)
            msq = sm.tile([128, KT], F32)
            s2col = sm.tile([128, KT], F32)
            lnd2 = sm.tile([128, KT], F32)
            dcol = sm.tile([128, KT], F32)
            dh32 = sm.tile([128, KT], F32)
            dlo = sm.tile([128, KT], F32)
            fd2s = [
                sm.tile([128, 2, 16], FP8, tag=f"fd2_{p}", name=f"fd2_{p}")
                for p in range(NP)
            ]
            for t in range(KT):
                diff = nscr.tile([128, D], BF16, tag="ascr")
                # Manual wait stamp: keep sub_t from being scheduled
                # ahead of chunk t-1's bn/aggr tail when the scheduler
                # mispredicts the n_t DMA (v2/v3 traces both showed the
                # DVE head-of-line blocked on a not-yet-landed n chunk).
                with tc.tile_wait_until(0.012 + 0.0042 * t):
                    nc.vector.tensor_sub(diff[:], n_ts[t][:], deep_b[:])
                for g in range(4):
                    nc.vector.bn_stats(
                        stats[:, t, g, :], diff[:, g * 512 : (g + 1) * 512]
                    )
                nc.vector.bn_aggr(mv[:, t, :], stats[:, t, :, :])
                # GpSimd: s2 = var + mean^2 (stamped: the v4 trace had
                # the scheduler park stage-B gpsimd ops ahead of these,
                # delaying d0 by ~7us)
                with tc.tile_wait_until(0.013 + 0.0042 * t):
                    nc.gpsimd.tensor_tensor(
                        msq[:, t : t + 1], mv[:, t, 0:1], mv[:, t, 0:1], OP.mult
                    )
                    nc.gpsimd.tensor_tensor(
                        s2col[:, t : t + 1], msq[:, t : t + 1], mv[:, t, 1:2], OP.add
                    )
                # ACT: d = sqrt(2048 * s2) = exp(0.5 ln(2048 * s2))
                nc.scalar.activation(
                    lnd2[:, t : t + 1], s2col[:, t : t + 1], AF.Ln, scale=float(D)
                )
                nc.scalar.activation(
                    dcol[:, t : t + 1], lnd2[:, t : t + 1], AF.Exp, scale=0.5
                )
                # GpSimd: fp8 hi/lo split (d is positive; w is -w)
                fd2 = fd2s[t // 2]
                c = t % 2
                with tc.tile_wait_until(0.0135 + 0.0042 * t):
                    nc.gpsimd.tensor_copy(fd2[:, c, 0:1], dcol[:, t : t + 1])
                    nc.gpsimd.tensor_copy(dh32[:, t : t + 1], fd2[:, c, 0:1])
                    nc.gpsimd.tensor_tensor(
                        dlo[:, t : t + 1],
                        dcol[:, t : t + 1],
                        dh32[:, t : t + 1],
                        OP.subtract,
                    )
                    nc.gpsimd.tensor_copy(fd2[:, c, 1:2], dlo[:, t : t + 1])

            # ---------------- stage B: local ce -----------------------
            ssum = sm.tile([128, KT], F32)
            for t in range(KT):
                escr = clsscr.tile([128, C], BF16, tag="bscr")
                nc.scalar.activation(
                    escr[:], clsb[:, t, :], AF.Exp, accum_out=ssum[:, t : t + 1]
                )
            lse = sm.tile([128, KT], F32)
            nc.scalar.activation(lse[:], ssum[:], AF.Ln)
            cecol = sm.tile([128, KT], F32)
            with tc.tile_wait_until(0.026):
                nc.gpsimd.tensor_tensor(cecol[:], lse[:], ncol_sb[:], OP.add)
            # DoubleRow lhsT pairs [ones | ce] per chunk pair, fp8
            sn2s = []
            for p in range(NP):
                sn2 = sm.tile([128, 2, 16], FP8, tag=f"sn2_{p}")
                nc.gpsimd.memset(sn2[:, :, 0:1], 1.0)
                with tc.tile_wait_until(0.0265):
                    nc.gpsimd.tensor_copy(
                        sn2[:, :, 1:2], cecol[:, 2 * p : 2 * p + 2]
                    )
                sn2s.append(sn2)

            # ------- stage C: sweep local wT over all W ---------------
            # One [34, W] f32 PSUM tile: rows 0-1 = [s, num] (DoubleRow,
            # must land at partition 0), rows 32-33 = [-f_hi, -f_lo]
            # (regular fp8, FD=1024 passes). exp pieces for chunks 2/3
            # are split so tail DR matmuls chase individual banks.
            e2s = [sm.tile([128, 2, W], FP8, tag=f"e2_{p}", name=f"e2_{p}") for p in range(NP)]

            # exp piece lists per chunk: (pair, c, lo, hi)
            pieces = {
                0: [(0, 0, 0, W)],
                1: [(0, 1, 0, W)],
                2: [(1, 0, 0, 2048), (1, 0, 2048, W)],
                3: [(1, 1, 0, 2048), (1, 1, 2048, 3072), (1, 1, 3072, W)],
            }
            for t in range(KT):
                for p, c, lo, hi in pieces[t]:
                    nc.scalar.activation(
                        e2s[p][:, c, lo:hi],
                        w2s[p][:, c, lo:hi],
                        AF.Exp,
                        scale=dcol[:, t : t + 1],
                    )

            # f matmuls: regular fp8, 512-wide passes, ALL accumulating
            # into the SAME [2, 512] psum region (host sums the 512
            # cols) so the tail copy is 512 cols instead of 4096.
            def f_mms(p, c):
                t = p * 2 + c
                for b in range(NB):
                    sl = slice(b * 512, (b + 1) * 512)
                    nc.tensor.matmul(
                        sn_psum[32:34, 0:512],
                        fd2s[p][:, c, 0:2],
                        w2s[p][:, c, sl],
                        start=(t == 0 and b == 0),
                        stop=(t == KT - 1 and b == NB - 1),
                        skip_group_check=True,
                    )

            def dr_mms(p, blo, bhi):
                for b in range(blo, bhi):
                    sl = slice(b * 512, (b + 1) * 512)
                    nc.tensor.matmul(
                        sn_psum[0:2, sl],
                        sn2s[p][:, :, 0:2],
                        e2s[p][:, :, sl],
                        start=(p == 0),
                        stop=(p == NP - 1),
                        perf_mode=DR,
                        skip_group_check=True,
                    )

            # PE program order: f chunks 0-2 early, DR pair A after
            # wexp1, f chunk 3 (gated on d3) overlapping wexp3, then DR
            # pair B banks chasing the wexp3 pieces.
            f_mms(0, 0)
            f_mms(0, 1)
            f_mms(1, 0)
            dr_mms(0, 0, NB)
            f_mms(1, 1)
            dr_mms(1, 0, 4)
            dr_mms(1, 4, 6)
            dr_mms(1, 6, 8)

            # PSUM -> SBUF (DMA cannot read PSUM). f finishes first ->
            # DVE takes its low half early, ACT the high half right
            # after the exp stream ends. sn copies chase DR banks in
            # quarters, split DVE/ACT.
            f_sb = sm.tile([2, 512], F32)
            nc.vector.tensor_copy(f_sb[:], sn_psum[32:34, 0:512])
            nc.sync.dma_start(out_d[2:4, 0:512], f_sb[:])
            sn_sb = sm.tile([2, W], F32)
            Q = W // 4
            nc.vector.tensor_copy(sn_sb[:, 0:Q], sn_psum[0:2, 0:Q])
            nc.vector.tensor_copy(sn_sb[:, Q : 2 * Q], sn_psum[0:2, Q : 2 * Q])
            nc.scalar.copy(sn_sb[:, 2 * Q : 3 * Q], sn_psum[0:2, 2 * Q : 3 * Q])
            nc.scalar.copy(sn_sb[:, 3 * Q : W], sn_psum[0:2, 3 * Q : W])
            nc.sync.dma_start(out_d[0:2, :], sn_sb[:])

    nc.compile()
    return nc


def _get_state():
    global _STATE
    if _STATE is None:
        _STATE = _build()
    return _STATE


def _shard_inputs(deep_feats, cls_score, target, n, w):
    import ml_dtypes

    bf16 = ml_dtypes.bfloat16
    fp8 = ml_dtypes.float8_e4m3
    deep_feats = np.ascontiguousarray(deep_feats, dtype=np.float32).reshape(1, D)
    cls_score = np.ascontiguousarray(cls_score, dtype=np.float32)
    n = np.ascontiguousarray(n, dtype=np.float32)
    w = np.ascontiguousarray(w, dtype=np.float32)
    tgt = int(np.asarray(target).reshape(-1)[0])
    ncol = -cls_score[:, tgt].astype(np.float32)

    deep_b = np.ascontiguousarray(np.broadcast_to(deep_feats.astype(bf16), (128, D)))
    n_bf = n.astype(bf16)
    cls_8 = cls_score.astype(fp8)
    wt_8 = np.ascontiguousarray((-w.T).astype(fp8))  # [K, W], negated

    in_maps = []
    for i in range(NCORES):
        ks = slice(i * KS, (i + 1) * KS)
        # cls reshaped so SBUF partition rows are KT*C bytes (4KB DMA
        # rows): cls_r[p, t*C + c] = cls[ks][t*128+p, c]
        cls_r = np.ascontiguousarray(
            cls_8[ks].reshape(KT, 128, C).transpose(1, 0, 2).reshape(128, KT * C)
        )
        in_maps.append(
            {
                "deep": deep_b,
                "n_s": n_bf[ks],
                "cls_s": cls_r,
                "ncol_s": ncol[ks],
                "wt_s": wt_8[ks],
            }
        )
    return in_maps


def _combine(outs):
    """Host-side unshard: sum the 8 [4, W] partials and finish the loss."""
    acc = np.zeros((4, W), dtype=np.float64)
    for o in outs:
        acc += np.asarray(o, dtype=np.float64)
    s_row, num_row = acc[0], acc[1]
    g = float(np.sum(num_row / s_row))
    f = -float(np.sum(acc[2, :512] + acc[3, :512]))  # rows hold -d*w partials
    return np.float32(g + f).reshape(())


def kernel(deep_feats, cls_score, target, n, w):
    nc = _get_state()
    from concourse.bass_utils import run_bass_kernel_spmd

    in_maps = _shard_inputs(deep_feats, cls_score, target, n, w)
    res = run_bass_kernel_spmd(nc, in_maps, list(range(NCORES)))
    return _combine([res.results[i]["out"] for i in range(NCORES)])
